# revision 2
# baseline (speedup 1.0000x reference)
"""Bahdanau attention Trainium2 kernel, v3 (bf16 main, improved PE schedule).

Problem shapes (fixed): B=64, T=1024, KS=QS=H=1024, fp32 in/out.
  proj_keys = keys @ W_key                  [B,T,H]
  q         = query @ W_query               [B,1,H]
  scores    = tanh(q + proj_keys) . w_score [B,T]
  alphas    = softmax(mask(scores))         [B,1,T]
  context   = alphas @ values               [B,1,KS]

Sharding: data-parallel over batch across 8 NeuronCores (8 batches/core),
weights replicated.

v3 changes vs v2 (sched="v3"):
  - score matmul of h-chunk m is deferred until after chunk m+1's main
    matmuls, so its tanh dependency resolves off the PE's critical path
    (PE head-of-queue stall removed; engines park only 4 blocked instrs)
  - the tail's alphasT PSUM->SBUF copy moved from ACT (queue deep in tanh
    work) to the idle DVE, unblocking the ctx matmuls

Retained from v2: bf16 matmul operands, DRAM->SBUF xbar DMA transpose for
keysT, softmax without max-subtraction with deferred normalization, batch
tail emitted inside the next batch's m-loop.

An fp8-e4m3 DoubleRow error-feedback main matmul (main_mode="fp8ef") is
implemented and numerically BETTER than bf16 (alph 3.4e-3 vs 4.8e-3 HW)
but measured 1.7x SLOWER (unhidden 256-col LDWEIGHTS per DR matmul), so
bf16 stays the default.
"""

import numpy as np

import concourse.bass as bass
import concourse.mybir as mybir
import concourse.tile as tile
from concourse.masks import make_identity

f32 = mybir.dt.float32
bf16 = mybir.dt.bfloat16
fp8 = mybir.dt.float8e4

# "bf16": v2 main matmul.  "fp8ef": error-feedback fp8 DoubleRow main matmul:
#   W' = 32*W;  Whi = q8(W'), Wlo = q8(W'-Whi), WhiQ4 = q8(Whi/4)
#   Khi = q8(K), Klo4 = q8(4*(K-Khi))
#   P' = [Whi;WhiQ4]路[Khi;Klo4] (pass A, 8 DR MMs: K captured to ~1e-3)
#      + [Wlo_2j;Wlo_2j+1]路[Khi_2j;Khi_2j+1] (pass B, 4 DR MMs: W captured)
#   tanh(P'/32 + q) via the ACT scale operand.  12 half-rate DR MMs replace
#   16 half-MM units of bf16 work -> 0.75x PE on the main matmul, with
#   BETTER precision than bf16 (proj relerr 1.2e-3 vs 2.3e-3).
MAIN_MODE = "bf16"

P = 128        # partitions
TB = 8         # batches per core
T = 1024       # sequence length
H = 1024       # hidden (= KS = QS)
NC_ = 8        # chunks of 128 along T/H/KS
NH = 512       # matmul moving free-dim (one PSUM bank of fp32)
TAIL_M = 2     # m-index in batch b+1's loop where batch b's tail is emitted

AX = mybir.AxisListType
ALU = mybir.AluOpType
ACT = mybir.ActivationFunctionType


def _split_drain_waits(nc, max_waits: int = 1):
    """walrus CTRL encoding supports a limited number of sem waits per
    instruction; Tile's final drain can carry many.  Hoist extras onto
    preceding single-wait drains."""
    for func in nc.m.functions:
        for blk in func.blocks:
            new_insts = []
            for inst in blk.instructions:
                si = inst.sync_info
                if si is not None and si.on_wait and len(si.on_wait) > max_waits:
                    waits = list(si.on_wait)
                    extra, keep = waits[:-max_waits], waits[-max_waits:]
                    for j, w in enumerate(extra):
                        new_insts.append(
                            mybir.InstDrain(
                                name=f"{inst.name}-presplit{j}",
                                engine=inst.engine,
                                sync_info=mybir.SyncInfo(on_wait=[w], on_update=[]),
                            )
                        )
                    si.on_wait = keep
                new_insts.append(inst)
            blk.instructions = new_insts


def build_bahdanau_nc(split_drains=True, reps=1, big_io=True, keys_mode="xbar_dram",
                      do_score=True, do_softmax=True, do_tail=True, main_mms=8,
                      main_mode=None, sched="v3"):
    """Build the per-core Bass program (identical on all 8 cores)."""
    import contextlib

    main_mode = main_mode or MAIN_MODE
    nc = bass.Bass(trn_type="TRN2", target_bir_lowering=False, debug=False)

    big = "ExternalInput" if big_io else "Internal"
    # keys/values/wkey are pre-cast to bf16 on the host (make_in_maps):
    # halves HBM traffic, drops all DVE rounds, and enables the proven
    # DRAM->SBUF xbar DMA transpose for keysT
    if main_mode == "fp8ef":
        # host-pretransposed keysT hi/lo planes: row (k*128+p) = (chunk k,
        # partition p); plane 0 = Khi e4m3, plane 1 = Klo4 e4m3
        keys8_d = nc.dram_tensor("keys8", [TB, H, 2, T], fp8, kind=big).ap()
        wk8hi_d = nc.dram_tensor("wk8hi", [H, 2, H], fp8, kind=big).ap()
        wk8lo_d = nc.dram_tensor("wk8lo", [H, H], fp8, kind=big).ap()
    else:
        keys_d = nc.dram_tensor("keys", [TB, T, H], bf16, kind=big).ap()
        wkey_d = nc.dram_tensor("wkey", [H, H], bf16, kind=big).ap()
    values_d = nc.dram_tensor("values", [TB, T, H], bf16, kind=big).ap()
    wquery_d = nc.dram_tensor("wquery", [H, H], f32, kind=big).ap()
    # queryt: host-prearranged query^T as [p, kchunk, b]
    qtin_d = nc.dram_tensor("qtin", [P, NC_, TB], f32, kind="ExternalInput").ap()
    # w_score host-prearranged as [p, kchunk]
    wsc_d = nc.dram_tensor("wsc", [P, NC_], f32, kind="ExternalInput").ap()
    # additive mask bias (0 where visible, -1e30 where masked)
    maskb_d = nc.dram_tensor("maskb", [TB, T], f32, kind="ExternalInput").ap()

    ctx_d = nc.dram_tensor("ctx", [TB, H], f32, kind="ExternalOutput").ap()
    alph_d = nc.dram_tensor("alph", [TB, T], f32, kind="ExternalOutput").ap()

    G = 4  # t/k-chunks per staging DMA

    with tile.TileContext(nc) as tc, contextlib.ExitStack() as ctx:
        # ---- pools
        const_pool = ctx.enter_context(tc.tile_pool(name="const", bufs=1))
        ktr_pool = ctx.enter_context(tc.tile_pool(name="ktr", bufs=3))
        s_pool = ctx.enter_context(tc.tile_pool(name="spool", bufs=2))
        v_pool = ctx.enter_context(tc.tile_pool(name="vpool", bufs=2))
        row_pool = ctx.enter_context(tc.tile_pool(name="rows", bufs=3))
        small_pool = ctx.enter_context(tc.tile_pool(name="small", bufs=2))
        knat_pool = ctx.enter_context(tc.tile_pool(name="knat", bufs=2))

        ps_pool = ctx.enter_context(tc.tile_pool(name="psS", bufs=2, space="PSUM"))
        sc_psum = ctx.enter_context(tc.tile_pool(name="scps", bufs=2, space="PSUM"))
        tl_psum = ctx.enter_context(tc.tile_pool(name="tlps", bufs=2, space="PSUM"))

        # ---- preamble
        ident = const_pool.tile([P, P], f32, tag="ident", name="ident")
        make_identity(nc, ident[:, :])
        identb = const_pool.tile([P, P], bf16, tag="identb", name="identb")
        nc.vector.tensor_copy(identb[:, :], ident[:, :])

        # prefetch ACT tables for Tanh/Exp during startup DMAs
        warm = const_pool.tile([1, 1], f32, tag="warm", name="warm")
        nc.scalar.activation(warm[:, :], ident[0:1, 0:1], ACT.Tanh)
        nc.scalar.activation(warm[:, :], ident[0:1, 0:1], ACT.Exp)

        # W_key: direct load into stationary tiles
        if main_mode == "fp8ef":
            wkhi2 = const_pool.tile([P, NC_, 2, H], fp8, tag="wk", name="wkhi2")
            nc.sync.dma_start(
                wkhi2[:, :, :, :],
                wk8hi_d[:, :, :].rearrange("(c p) two h -> p c two h", p=P),
            )
            wklo = const_pool.tile([P, NC_, H], fp8, tag="wklo", name="wklo")
            nc.sync.dma_start(
                wklo[:, :, :], wk8lo_d[:, :].rearrange("(c p) h -> p c h", p=P)
            )
        else:
            wk = const_pool.tile([P, NC_, H], bf16, tag="wk", name="wk")
            nc.sync.dma_start(
                wk[:, :, :], wkey_d[:, :].rearrange("(c p) h -> p c h", p=P)
            )

        qtin = const_pool.tile([P, NC_, TB], f32, tag="qtin", name="qtin")
        nc.sync.dma_start(qtin[:, :, :], qtin_d[:, :, :])
        wsc_raw = const_pool.tile([P, NC_], f32, tag="wsc_raw", name="wsc_raw")
        nc.sync.dma_start(wsc_raw[:, :], wsc_d[:, :])
        wsc = const_pool.tile([P, NC_], bf16, tag="wsc", name="wsc")
        nc.vector.tensor_copy(wsc[:, :], wsc_raw[:, :])
        # W_query: staged through the knat ring (dead after the q projection)
        wq4 = []
        for g in range(2):
            wqt = knat_pool.tile([P, G, H], f32, tag="knat", name=f"wq{g}")
            nc.sync.dma_start(
                wqt[:, :, :],
                wquery_d[g * G * P : (g + 1) * G * P, :].rearrange(
                    "(c p) h -> p c h", p=P
                ),
            )
            wq4.append(wqt)
        qT = const_pool.tile([P, NC_, TB], f32, tag="qT", name="qT")

        def emit_keys(b, ktr_dst):
            """keysT via 8 DRAM->SBUF xbar DMA transposes (contiguous source
            column-slab and contiguous [128,1024] destination -- the pattern
            tile_matmul.py uses on HW), or v1-style PE transposes in bf16."""
            if main_mode == "fp8ef":
                # host-pretransposed hi/lo fp8 planes: 2 plain strided DMAs
                for g in range(2):
                    nc.sync.dma_start(
                        ktr_dst[:, g * 4 : (g + 1) * 4, :, :],
                        keys8_d[b, g * 4 * P : (g + 1) * 4 * P, :, :].rearrange(
                            "(c p) two t -> p c two t", p=P
                        ),
                    )
                return
            if keys_mode == "xbar_dram":
                for k in range(NC_):
                    nc.sync.dma_start_transpose(
                        ktr_dst[:, k, :], keys_d[b, :, k * P : (k + 1) * P]
                    )
                return
            for g in range(2):
                knb = knat_pool.tile([P, G, H], bf16, tag="knb", name=f"knb{b}_{g}")
                nc.sync.dma_start(
                    knb[:, :, :],
                    keys_d[b, g * G * P : (g + 1) * G * P, :].rearrange(
                        "(c p) h -> p c h", p=P
                    ),
                )
                for j in range(G):
                    t = g * G + j
                    if True:
                        for h in range(2):
                            ptr = tl_psum.tile(
                                [P, 4 * P], bf16, tag="tl", name=f"ptr{b}_{t}_{h}"
                            )
                            for jj in range(4):
                                k = 4 * h + jj
                                nc.tensor.transpose(
                                    ptr[:, jj * P : (jj + 1) * P],
                                    knb[:, j, k * P : (k + 1) * P],
                                    identb[:, :],
                                )
                            src_ = ptr[:, :].rearrange("p (k c) -> p k c", k=4)
                            dst = ktr_dst[:, 4 * h : 4 * h + 4, t * P : (t + 1) * P]
                            if h == 0:
                                nc.vector.tensor_copy(dst, src_)
                            else:
                                nc.scalar.copy(dst, src_)

        # ---- steady-state batch pipeline (reps>1 repeats for timing only)
        for rep in range(reps):
            ktr_shape = [P, NC_, 2, T] if main_mode == "fp8ef" else [P, NC_, T]
            ktr_dt = fp8 if main_mode == "fp8ef" else bf16

            mb_cur = small_pool.tile([1, T], f32, tag="mb", name=f"mb_r{rep}b0")
            nc.sync.dma_start(mb_cur[:, :], maskb_d[0:1, :])
            ktrs = {0: ktr_pool.tile(ktr_shape, ktr_dt, tag="ktr", name=f"ktr_r{rep}b0")}
            emit_keys(0, ktrs[0])
            if TB > 1:
                ktrs[1] = ktr_pool.tile(
                    ktr_shape, ktr_dt, tag="ktr", name=f"ktr_r{rep}b1"
                )
                emit_keys(1, ktrs[1])
            ktr_cur = ktrs[0]

            if rep == 0:
                # q projection (exact fp32); all 64 [h,b] columns accumulate
                # in one PSUM tile borrowed from the ps ring
                psq = ps_pool.tile([P, T], f32, tag="ps", name="psq")
                for m in range(NC_):
                    for k in range(NC_):
                        nc.tensor.matmul(
                            psq[:, m * TB : (m + 1) * TB],
                            lhsT=wq4[k // G][:, k % G, m * P : (m + 1) * P],
                            rhs=qtin[:, k, :],
                            start=(k == 0),
                            stop=(k == NC_ - 1),
                        )
                nc.scalar.copy(
                    qT[:, :, :],
                    psq[:, 0 : NC_ * TB].rearrange("p (m b) -> p m b", m=NC_),
                )

            pending = None  # batch whose PE tail (alpha transposes+ctx) is due

            def emit_tail(pb, arow_exp, rinv, vts):
                """PE tail of batch pb: alphas row->cols, ctx matmuls, drains."""
                paT = tl_psum.tile([P, TB], f32, tag="tl", name=f"paT{rep}_{pb}")
                for k in range(NC_):
                    nc.tensor.transpose(
                        paT[:, k : k + 1],
                        arow_exp[0:1, k * P : (k + 1) * P],
                        ident[0:1, 0:1],
                    )
                aT = small_pool.tile([P, NC_], bf16, tag="aT", name=f"aT{rep}_{pb}")
                if sched == "v3":
                    # DVE is idle; ACT's queue is deep in tanh work, so a
                    # scalar.copy here stalls the PE's ctx matmuls behind it
                    nc.vector.tensor_copy(aT[:, :], paT[:, :])
                else:
                    nc.scalar.copy(aT[:, :], paT[:, :])

                pcx = [
                    tl_psum.tile([1, NH], f32, tag="tl", name=f"pcx{rep}_{pb}_{n}")
                    for n in range(2)
                ]
                for k in range(NC_):
                    for n in range(2):
                        nc.tensor.matmul(
                            pcx[n][:, :],
                            lhsT=aT[:, k : k + 1],
                            rhs=vts[k // G][:, k % G, n * NH : (n + 1) * NH],
                            start=(k == 0),
                            stop=(k == NC_ - 1),
                        )
                # drain with deferred softmax normalization: ctx = pcx * rinv
                cxr = row_pool.tile([1, T], f32, tag="row", name=f"cxr{rep}_{pb}")
                for n in range(2):
                    nc.scalar.mul(
                        cxr[:, n * NH : (n + 1) * NH], pcx[n][:, :], rinv[:, :]
                    )
                nc.sync.dma_start(ctx_d[pb : pb + 1, :], cxr[0:1, :H])

            for b in range(TB):
                last = b == TB - 1
                if not last:
                    mb_next = small_pool.tile(
                        [1, T], f32, tag="mb", name=f"mb_r{rep}b{b + 1}"
                    )
                    nc.sync.dma_start(mb_next[:, :], maskb_d[b + 1 : b + 2, :])
                    if b + 2 < TB:
                        ktrs[b + 2] = ktr_pool.tile(
                            ktr_shape, ktr_dt, tag="ktr", name=f"ktr_r{rep}b{b + 2}"
                        )
                        emit_keys(b + 2, ktrs[b + 2])

                # values prefetch: 2 direct bf16 DMAs (consumed by the
                # ctx matmul one batch later -- bufs=4 keeps 2 batches)
                vts = []
                for g in range(2):
                    vt = v_pool.tile(
                        [P, G, H], bf16, tag="v", bufs=4, name=f"v{rep}_{b}_{g}"
                    )
                    nc.sync.dma_start(
                        vt[:, :, :],
                        values_d[b, g * G * P : (g + 1) * G * P, :].rearrange(
                            "(c p) h -> p c h", p=P
                        ),
                    )
                    vts.append(vt)

                # main matmul + tanh + scores
                psc = [
                    sc_psum.tile([1, NH], f32, tag="sm", name=f"psc{rep}_{b}_{n}")
                    for n in range(2)
                ]
                def emit_score(sm, stile):
                    for n in range(2):
                        nc.tensor.matmul(
                            psc[n][:, :],
                            lhsT=wsc[:, sm : sm + 1],
                            rhs=stile[:, n * NH : (n + 1) * NH],
                            start=(sm == 0),
                            stop=(sm == NC_ - 1),
                        )

                pending_score = None
                for m in range(NC_):
                    ps = ps_pool.tile([P, T], f32, tag="ps", name=f"ps{rep}_{b}_{m}")
                    if main_mode == "fp8ef":
                        DR = mybir.MatmulPerfMode.DoubleRow
                        # pass A: [Whi;WhiQ4] . [Khi;Klo4] -> K@Whi (K-capture)
                        for k in range(NC_):
                            for n in range(2):
                                nc.tensor.matmul(
                                    ps[:, n * NH : (n + 1) * NH],
                                    lhsT=wkhi2[:, k, :, m * P : (m + 1) * P],
                                    rhs=ktr_cur[:, k, :, n * NH : (n + 1) * NH],
                                    start=(k == 0),
                                    stop=False,
                                    perf_mode=DR,
                                )
                        # pass B: [Wlo_2j;Wlo_2j+1] . [Khi_2j;Khi_2j+1]
                        for j in range(NC_ // 2):
                            for n in range(2):
                                nc.tensor.matmul(
                                    ps[:, n * NH : (n + 1) * NH],
                                    lhsT=wklo[:, 2 * j : 2 * j + 2, m * P : (m + 1) * P],
                                    rhs=ktr_cur[:, 2 * j : 2 * j + 2, 0, n * NH : (n + 1) * NH],
                                    start=False,
                                    stop=(j == NC_ // 2 - 1),
                                    perf_mode=DR,
                                )
                    else:
                        for k in range(main_mms):
                            for n in range(2):
                                nc.tensor.matmul(
                                    ps[:, n * NH : (n + 1) * NH],
                                    lhsT=wk[:, k, m * P : (m + 1) * P],
                                    rhs=ktr_cur[:, k, n * NH : (n + 1) * NH],
                                    start=(k == 0),
                                    stop=(k == main_mms - 1),
                                )
                    if do_score and sched == "v3":
                        # score MM of chunk m-1: its tanh finished during
                        # chunk m's main MMs -> no PE head-of-queue stall
                        if pending_score is not None:
                            emit_score(*pending_score)
                            pending_score = None
                        s = s_pool.tile([P, T], bf16, tag="s", name=f"s{rep}_{b}_{m}")
                        nc.scalar.activation(
                            s[:, :], ps[:, :], ACT.Tanh, bias=qT[:, m, b : b + 1],
                            scale=(1.0 / 32.0) if main_mode == "fp8ef" else 1.0,
                        )
                        pending_score = (m, s)
                    elif do_score:
                        s = s_pool.tile([P, T], bf16, tag="s", name=f"s{rep}_{b}_{m}")
                        nc.scalar.activation(
                            s[:, :], ps[:, :], ACT.Tanh, bias=qT[:, m, b : b + 1],
                            scale=(1.0 / 32.0) if main_mode == "fp8ef" else 1.0,
                        )
                        emit_score(m, s)
                    if m == TAIL_M and pending is not None:
                        emit_tail(*pending)
                        pending = None
                if do_score and pending_score is not None:
                    emit_score(*pending_score)
                    pending_score = None

                if do_softmax and do_score:
                    # softmax head (ACT/DVE only; no max-subtraction -- scores
                    # are bounded by |w_score|_1 ~ 26, exp is safe in fp32)
                    sc = row_pool.tile([1, T], f32, tag="row", name=f"sc{rep}_{b}")
                    for n in range(2):
                        nc.vector.tensor_add(
                            sc[:, n * NH : (n + 1) * NH],
                            psc[n][:, :],
                            mb_cur[:, n * NH : (n + 1) * NH],
                        )
                    arow_exp = row_pool.tile([1, T], f32, tag="row", name=f"ae{rep}_{b}")
                    ssum = small_pool.tile([1, 1], f32, tag="ssum", name=f"ssum{rep}_{b}")
                    nc.scalar.activation(
                        arow_exp[:, :], sc[:, :], ACT.Exp, accum_out=ssum[:, :]
                    )
                    rinv = small_pool.tile([1, 1], f32, tag="rinv", name=f"rinv{rep}_{b}")
                    nc.vector.reciprocal(rinv[:, :], ssum[:, :])
                    # normalized alphas row out (off critical path)
                    arow_n = row_pool.tile([1, T], f32, tag="rowo", name=f"an{rep}_{b}")
                    nc.vector.tensor_scalar_mul(arow_n[:, :], arow_exp[:, :], rinv[:, :])
                    nc.sync.dma_start(alph_d[b : b + 1, :], arow_n[:, :])

                    if do_tail:
                        pending = (b, arow_exp, rinv, vts)
                if not last:
                    ktr_cur = ktrs[b + 1]
                    mb_cur = mb_next
                del ktrs[b]
            if pending is not None:
                emit_tail(*pending)

    if split_drains:
        _split_drain_waits(nc)
    return nc


_NC_CACHE = None


def _get_nc():
    global _NC_CACHE
    if _NC_CACHE is None:
        _NC_CACHE = build_bahdanau_nc()
    return _NC_CACHE


def make_in_maps(query, mask, values, keys, W_key, W_query, w_score,
                 main_mode=None):
    """Shard full inputs into per-core input maps (host-side layout +
    bf16/fp8 pre-cast of the large tensors)."""
    import ml_dtypes

    main_mode = main_mode or MAIN_MODE
    bf = ml_dtypes.bfloat16
    e4 = ml_dtypes.float8_e4m3
    query = np.ascontiguousarray(np.asarray(query, dtype=np.float32))
    mask = np.asarray(mask)
    values = np.ascontiguousarray(np.asarray(values, dtype=np.float32).astype(bf))
    keys = np.asarray(keys, dtype=np.float32)
    W_key = np.asarray(W_key, dtype=np.float32)
    W_query = np.ascontiguousarray(np.asarray(W_query, dtype=np.float32))
    w_score = np.ascontiguousarray(np.asarray(w_score, dtype=np.float32))

    B = query.shape[0]
    n_cores = B // TB
    maskb = np.where(mask, np.float32(0.0), np.float32(-1e30)).astype(np.float32)
    wsc_in = np.ascontiguousarray(w_score.reshape(NC_, P).T)

    if main_mode == "fp8ef":
        def q8(x):
            return np.clip(x, -240, 240).astype(e4)

        Khi = q8(keys)
        Klo4 = q8(4.0 * (keys - Khi.astype(np.float32)))
        # keysT hi/lo planes: [B, KS, 2, T]
        keys8 = np.empty((B, H, 2, T), dtype=e4)
        keys8[:, :, 0, :] = Khi.transpose(0, 2, 1)
        keys8[:, :, 1, :] = Klo4.transpose(0, 2, 1)
        keys8 = np.ascontiguousarray(keys8)
        Wp = 32.0 * W_key
        Whi = q8(Wp)
        Wlo = q8(Wp - Whi.astype(np.float32))
        WhiQ4 = q8(Whi.astype(np.float32) / 4.0)
        wk8hi = np.empty((H, 2, H), dtype=e4)
        wk8hi[:, 0, :] = Whi
        wk8hi[:, 1, :] = WhiQ4
        wk8hi = np.ascontiguousarray(wk8hi)
        wk8lo = np.ascontiguousarray(Wlo)
    else:
        keys_bf = np.ascontiguousarray(keys.astype(bf))
        wkey_bf = np.ascontiguousarray(W_key.astype(bf))

    in_maps = []
    for c in range(n_cores):
        sl = slice(c * TB, (c + 1) * TB)
        qt = query[sl, 0, :].T  # [QS, TB]
        qtin = np.ascontiguousarray(qt.reshape(NC_, P, TB).transpose(1, 0, 2))
        m = {
            "values": values[sl],
            "wquery": W_query,
            "qtin": qtin,
            "wsc": wsc_in,
            "maskb": np.ascontiguousarray(maskb[sl]),
        }
        if main_mode == "fp8ef":
            m["keys8"] = keys8[sl]
            m["wk8hi"] = wk8hi
            m["wk8lo"] = wk8lo
        else:
            m["keys"] = keys_bf[sl]
            m["wkey"] = wkey_bf
        in_maps.append(m)
    return in_maps


def kernel(query, mask, values, keys, W_key, W_query, w_score):
    from concourse.bass_utils import run_bass_kernel_spmd

    B = np.asarray(query).shape[0]
    n_cores = B // TB
    in_maps = make_in_maps(query, mask, values, keys, W_key, W_query, w_score)
    nc = _get_nc()
    try:
        res = run_bass_kernel_spmd(nc, in_maps, core_ids=list(range(n_cores)))
    except Exception:
        # transient NRT_EXEC_UNIT_UNRECOVERABLE wedges have been observed to
        # clear on retry
        import time as _time

        _time.sleep(2.0)
        res = run_bass_kernel_spmd(nc, in_maps, core_ids=list(range(n_cores)))
    context = np.concatenate([r["ctx"] for r in res.results], axis=0)
    alphas = np.concatenate([r["alph"] for r in res.results], axis=0)
    return context.reshape(B, 1, H), alphas.reshape(B, 1, T)

